# revision 15
# baseline (speedup 1.0000x reference)
"""TRN2 Bass kernel for nn_BWSGRicker: T=10000-step 4-variable Ricker recurrence.

Strategy: the recurrence s_{k+1} = min(s_k * exp(u(s_k)) + mask_k, 1) is solved
by chunked Picard (waveform relaxation) iteration instead of 10000 serial steps:
  - guess a trajectory for a chunk of steps (zeros / previous iterate),
  - compute all growth factors E_k = exp(u(s_k)) for the chunk vectorized
    (ACT exp for eS, 2 PE matmuls, 2 DVE elementwise, ACT exp),
  - run tensor_tensor_scan (state = min(E*state, 1)) to re-propagate the chunk,
  - repeat until the chunk is a bitwise fixed point, then move on.
The dynamics contract strongly for the target inputs (trajectory collapses to
denormal dust within ~300 steps), so a few sweeps converge the transient
exactly, and the tail is verified in one shot: for the all-zeros tail guess the
growth factors are one constant vector E0 (plus the boundary column), so a
single long scan with a stride-0 broadcast of E0 re-propagates and verifies the
entire tail. Convergence is *proved* on device: each chunk's final sweep writes
to a fresh buffer and max |delta| vs the previous iterate accumulates into a
flag output; flag <= 1e-35 (denormal dust) guarantees the result is the fixed
point of the device step map to within denormal noise. Otherwise the host
re-runs a warm-started refinement program until the flag is clean.

Sharding: a single trajectory is a strict sequential recurrence on 4 scalars --
it cannot be sharded. All 8 cores run the same program (SPMD replication);
core 0's output is returned.
"""
import numpy as np

T = 10000
TAIL_START = 513          # cold-start schedule: full sweeps before, one-shot after
CM = 128                  # max full-sweep chunk width (cold schedule)
CM_WARM = 1024            # max chunk width (warm refinement schedule)
TP = T + 16
NP_OUT = 125              # output staging: 10000 rows = 125 partitions x 80
RPP = 80
CONV_EPS = 1e-35


def _schedule_normal():
    # (1,257) converges in 6 sweeps in the host model; 7 leaves a margin for
    # device-ulp differences. The flag check still proves the result.
    return [(1, 257, 7), (257, 513, 2)]


def _schedule_warm():
    sched = [(1, 513, 6)]
    a = 513
    while a < T:
        b = min(a + CM_WARM, T)
        sched.append((a, b, 3))
        a = b
    return sched


def _host_constants(y0, params, logpsi):
    f = np.float32
    y0 = np.asarray(y0, f)
    p = np.asarray(params, f)
    lp = np.asarray(logpsi, f)
    ep0 = np.exp(lp[0], dtype=f)
    ep2 = np.exp(lp[2], dtype=f)
    ep3 = np.exp(lp[3], dtype=f)
    ep4 = np.exp(lp[4], dtype=f)
    ep5 = np.exp(lp[5], dtype=f)
    ep6 = np.exp(lp[6], dtype=f)
    psi1 = lp[1]
    # lane order [S, W, B, G]
    s0 = np.array([y0[2], y0[1], y0[0], y0[3]], f)
    # Wa: [S,W,B,G,1] -> -q   (u = -q*s + (q+r))
    Wa = np.zeros((5, 4), f)
    Wa[3, 0] = -p[0]
    Wa[0, 1] = -p[1]
    Wa[0, 2] = -p[2]
    Wa[1, 2] = -p[2]
    Wa[4, 3] = -ep0
    # Wbc: [S,W,B,G,1, 0*27, eS] -> q + r  (tall K=33; rows 5..31 zeros)
    Wbc = np.zeros((33, 4), f)
    Wbc[3, 0] = p[0]
    Wbc[1, 0] = -ep2
    Wbc[2, 0] = -ep2
    Wbc[4, 0] = -ep3
    Wbc[0, 1] = p[1]
    Wbc[2, 1] = -ep4
    Wbc[4, 1] = -ep5
    Wbc[0, 2] = p[2]
    Wbc[1, 2] = p[2]
    Wbc[4, 2] = -ep6
    Wbc[4, 3] = ep0
    Wbc[32, 3] = -1.0           # -eS into uG
    # mask position: reference masks step j where j == (10.0 + i) - 1.0 in f32
    i_val = f(y0[4])
    jstar = None
    if i_val != 0.0:
        tgt = f(f(f(10.0) + i_val) - f(1.0))
        js = np.arange(1, T, dtype=f)
        hits = np.nonzero(js == tgt)[0]
        if len(hits):
            jstar = int(hits[0]) + 1
    maskv = np.zeros((4, 1), f)
    maskv[2, 0] = f(10.0 / 400.0)  # lane B is index 2
    # growth factors of the all-zeros state, lane order [S,W,B,G]; if all < 1
    # (with margin covering device-matmul ulps) the zero-tail verify chain is
    # monotone decreasing, so only its first column needs to be checked.
    e0 = np.exp(np.array([-ep3, -ep5, -ep6, ep0 - f(1.0)], f), dtype=f)
    return dict(s0=s0, Wa=Wa, Wbc=Wbc, psi1=float(psi1),
                jstar=jstar, maskv=maskv, e0_lt1=bool((e0 < 0.99).all()))


def _build_program(consts, schedule, tail_start):
    import concourse.bacc as bacc
    import concourse.mybir as mybir
    from concourse import tile

    f32 = mybir.dt.float32
    MULT = mybir.AluOpType.mult
    ADD = mybir.AluOpType.add
    SUB = mybir.AluOpType.subtract
    MAXOP = mybir.AluOpType.max
    MIN = mybir.AluOpType.min
    EXP = mybir.ActivationFunctionType.Exp
    X = mybir.AxisListType.X

    jstar = consts["jstar"]
    cm = max(b - a for (a, b, _s) in schedule)
    tail_n = (T - tail_start) if tail_start is not None else 0

    nc = bacc.Bacc("TRN2", target_bir_lowering=False, debug=False, num_devices=8)
    init_in = nc.dram_tensor("binit", [5, TP], f32, kind="ExternalInput").ap()
    wa_in = nc.dram_tensor("wa", [5, 4], f32, kind="ExternalInput").ap()
    wbc_in = nc.dram_tensor("wbc", [33, 4], f32, kind="ExternalInput").ap()
    mask_in = nc.dram_tensor("maskv", [4, 1], f32, kind="ExternalInput").ap()
    y_out = nc.dram_tensor("y", [T, 5], f32, kind="ExternalOutput").ap()
    flag_out = nc.dram_tensor("flag", [4, 1], f32, kind="ExternalOutput").ap()

    with tile.TileContext(nc) as tc:
        with tc.tile_pool(name="p", bufs=1) as pool, \
             tc.tile_pool(name="ps", bufs=1, space="PSUM") as psum_pool:
            # rows 0-3: trajectory [S,W,B,G]; row 4: ones; 5-31: zeros; 32: eS
            Big = pool.tile([33, TP], f32)
            Wa = pool.tile([5, 4], f32)
            Wbc = pool.tile([33, 4], f32)
            MK = pool.tile([4, 1], f32)
            ON1 = pool.tile([4, 1], f32)
            Tq = pool.tile([4, cm], f32)
            Uu = pool.tile([4, cm], f32)
            E = pool.tile([4, cm], f32)
            NewT = pool.tile([4, cm], f32)
            D = pool.tile([4, cm], f32)
            Dm = pool.tile([4, 1], f32)
            CI = pool.tile([4, 1], f32)
            JT = pool.tile([4, 1], f32)
            FlagAcc = pool.tile([4, 1], f32)
            Y4 = pool.tile([NP_OUT, 4 * RPP], f32)
            Yb = pool.tile([NP_OUT, 5 * RPP], f32)
            if tail_n:
                NTT = pool.tile([4, tail_n], f32)
            Vq = psum_pool.tile([4, cm], f32)
            Vqr = psum_pool.tile([4, cm], f32)

            # zero the garbage rows 5..31 over every column the K=33 matmul
            # will read; trajectory zeros beyond come from the binit DMA.
            zspan = max(b for (_a, b, _s) in schedule) + 8
            nc.vector.memset(Big[0:32, 0:zspan], 0.0)
            nc.sync.dma_start(Big[0:5, :], init_in[:])
            nc.sync.dma_start(Wa[:], wa_in[:])
            nc.sync.dma_start(Wbc[:], wbc_in[:])
            nc.sync.dma_start(MK[:], mask_in[:])
            nc.vector.memset(ON1[:], 1.0)
            nc.vector.memset(FlagAcc[:], 0.0)
            nc.vector.memset(Yb[:], 0.0)

            def ones_b(n):
                return ON1[:, 0:1].broadcast_to((4, n))

            def emit_scan(dest, dcol, n, a, Esrc, ecol=0):
                """dest[:, dcol:dcol+n] = scan of steps [a, a+n) with factors
                Esrc[:, ecol:ecol+n]; init from Big col a-1. Mask split."""
                init_ap = Big[0:4, a - 1:a]
                if jstar is not None and a <= jstar < a + n:
                    L1 = jstar - a
                    if L1 > 0:
                        nc.vector.tensor_tensor_scan(
                            dest[0:4, dcol:dcol + L1],
                            Esrc[:, ecol:ecol + L1], ones_b(L1),
                            init_ap, MULT, MIN)
                        prev_ap = dest[0:4, dcol + L1 - 1:dcol + L1]
                    else:
                        prev_ap = init_ap
                    nc.vector.tensor_tensor(CI[:], Esrc[:, ecol + L1:ecol + L1 + 1],
                                            prev_ap, MULT)
                    nc.vector.scalar_tensor_tensor(
                        dest[0:4, dcol + L1:dcol + L1 + 1], CI[:], MK[:],
                        ON1[:, 0:1], ADD, MIN)
                    if L1 + 1 < n:
                        nc.vector.tensor_tensor_scan(
                            dest[0:4, dcol + L1 + 1:dcol + n],
                            Esrc[:, ecol + L1 + 1:ecol + n], ones_b(n - L1 - 1),
                            dest[0:4, dcol + L1:dcol + L1 + 1], MULT, MIN)
                else:
                    nc.vector.tensor_tensor_scan(
                        dest[0:4, dcol:dcol + n], Esrc[:, ecol:ecol + n],
                        ones_b(n), init_ap, MULT, MIN)

            def emit_features(cs, ce):
                """E[:, 0:ce-cs] = growth factors for input-state cols [cs,ce)."""
                n = ce - cs
                nc.scalar.activation(Big[32:33, cs:ce], Big[0:1, cs:ce],
                                     EXP, scale=consts["psi1"])
                for o in range(0, n, 512):
                    e = min(o + 512, n)
                    nc.tensor.matmul(Vq[:, o:e], Wa[:],
                                     Big[0:5, cs + o:cs + e],
                                     start=True, stop=True)
                    nc.tensor.matmul(Vqr[:, o:e], Wbc[:],
                                     Big[0:33, cs + o:cs + e],
                                     start=True, stop=True)
                nc.vector.tensor_tensor(Tq[:, 0:n], Vq[:, 0:n],
                                        Big[0:4, cs:ce], MULT)
                nc.vector.tensor_tensor(Uu[:, 0:n], Tq[:, 0:n],
                                        Vqr[:, 0:n], ADD)
                nc.scalar.activation(E[:, 0:n], Uu[:, 0:n], EXP)
                nc.vector.tensor_copy(JT[:], E[:, 0:1])  # wait-absorbing joiner

            for (a, b, sweeps) in schedule:
                n = b - a
                for s in range(sweeps):
                    last = s == sweeps - 1
                    emit_features(a - 1, b - 1)
                    if not last:
                        nc.vector.tensor_copy(Big[0:4, a:a + 1], E[:, 0:1])
                        emit_scan(Big, a, n, a, E)
                    else:
                        emit_scan(NewT, 0, n, a, E)
                        nc.vector.tensor_tensor(D[:, 0:n], NewT[:, 0:n],
                                                Big[0:4, a:b], SUB)
                        nc.vector.tensor_reduce(Dm[:], D[:, 0:n], X, MAXOP,
                                                apply_absolute_value=True)
                        nc.vector.tensor_tensor(FlagAcc[:], FlagAcc[:], Dm[:],
                                                MAXOP)

            if tail_n:
                # One-shot verification of the all-zeros tail guess: states are
                # zero everywhere except the boundary, so the growth factors
                # are [E_boundary, E0, E0, ...]. Two columns of real feature
                # math give both.
                ts0 = tail_start
                tiny = consts["e0_lt1"] and (jstar is None or jstar < ts0)
                emit_features(ts0 - 1, ts0 + 1)   # E[:,0]=E_ts0, E[:,1]=E0
                # col ts0 result
                nc.vector.tensor_tensor(CI[:], E[:, 0:1],
                                        Big[0:4, ts0 - 1:ts0], MULT)
                if jstar is not None and jstar == ts0:
                    nc.vector.scalar_tensor_tensor(NTT[0:4, 0:1], CI[:], MK[:],
                                                   ON1[:, 0:1], ADD, MIN)
                else:
                    nc.vector.tensor_scalar_min(NTT[0:4, 0:1], CI[:], 1.0)
                if tiny:
                    # E0 < 1 in every lane (host-verified with margin), so the
                    # verify chain min(E0*s, 1) decreases monotonically: its
                    # max |.| is the first column.
                    nc.vector.tensor_reduce(Dm[:], NTT[0:4, 0:1], X, MAXOP,
                                            apply_absolute_value=True)
                else:
                    # full re-propagation with stride-0 E0 factors
                    rem = tail_n - 1
                    a2 = ts0 + 1
                    E0b = E[:, 1:2]
                    init2 = NTT[0:4, 0:1]
                    if jstar is not None and a2 <= jstar < T:
                        L1 = jstar - a2
                        if L1 > 0:
                            nc.vector.tensor_tensor_scan(
                                NTT[0:4, 1:1 + L1], E0b.broadcast_to((4, L1)),
                                ones_b(L1), init2, MULT, MIN)
                            prev_ap = NTT[0:4, L1:L1 + 1]
                        else:
                            prev_ap = init2
                        nc.vector.tensor_tensor(CI[:], E0b, prev_ap, MULT)
                        nc.vector.scalar_tensor_tensor(
                            NTT[0:4, 1 + L1:2 + L1], CI[:], MK[:],
                            ON1[:, 0:1], ADD, MIN)
                        if L1 + 1 < rem:
                            nc.vector.tensor_tensor_scan(
                                NTT[0:4, 2 + L1:tail_n],
                                E0b.broadcast_to((4, rem - L1 - 1)),
                                ones_b(rem - L1 - 1),
                                NTT[0:4, 1 + L1:2 + L1], MULT, MIN)
                    elif rem > 0:
                        nc.vector.tensor_tensor_scan(
                            NTT[0:4, 1:tail_n], E0b.broadcast_to((4, rem)),
                            ones_b(rem), init2, MULT, MIN)
                    nc.vector.tensor_reduce(Dm[:], NTT[0:4, 0:tail_n], X,
                                            MAXOP, apply_absolute_value=True)
                nc.vector.tensor_tensor(FlagAcc[:], FlagAcc[:], Dm[:], MAXOP)

            # output: y[t] = [B, W, S, G, 0] = lanes [2, 1, 0, 3] + zeros.
            # Two-hop rearrangement so every DMA burst is >= 320 bytes.
            for c, lane in ((0, 2), (1, 1), (2, 0), (3, 3)):
                nc.sync.dma_start(Y4[:, RPP * c:RPP * (c + 1)],
                                  Big[lane:lane + 1, 0:T])
            for c in range(4):
                nc.vector.tensor_copy(Yb[:, c:5 * RPP:5],
                                      Y4[:, RPP * c:RPP * (c + 1)])
            nc.sync.dma_start(
                y_out.rearrange("(p r) c -> p (r c)", p=NP_OUT), Yb[:])
            nc.sync.dma_start(flag_out[:], FlagAcc[:])

    nc.finalize()
    return nc


LAST_RESULTS = None  # BassKernelResults of the most recent device run


def _run(consts, schedule, warm_init, tail_start):
    import os
    from concourse.bass_utils import run_bass_kernel_spmd

    nc = _build_program(consts, schedule, tail_start)
    binit = np.zeros((5, TP), np.float32)
    binit[4, :] = 1.0
    if warm_init is not None:
        binit[0:4, 0:T] = warm_init
    binit[0:4, 0] = consts["s0"]
    maskv = consts["maskv"] if consts["jstar"] is not None \
        else np.zeros((4, 1), np.float32)
    im = {"binit": binit, "wa": consts["Wa"], "wbc": consts["Wbc"],
          "maskv": maskv}
    trace = bool(os.environ.get("BWSG_TRACE"))
    r = run_bass_kernel_spmd(nc, [im] * 8, list(range(8)), trace=trace)
    global LAST_RESULTS
    LAST_RESULTS = r
    res = r.results
    y = np.asarray(res[0]["y"])
    flag = float(np.asarray(res[0]["flag"]).max())
    return y, flag


def kernel(y0, params, logpsi):
    consts = _host_constants(y0, params, logpsi)
    y, flag = _run(consts, _schedule_normal(), None, TAIL_START)
    tries = 0
    while flag > CONV_EPS and tries < 8:
        # warm-started refinement until the device proves a fixed point
        warm = y[:, [2, 1, 0, 3]].T.copy()  # back to lane order [S,W,B,G]
        y, flag = _run(consts, _schedule_warm(), warm, None)
        tries += 1
    return y.astype(np.float32)


# revision 16
# speedup vs baseline: 1.1453x; 1.1453x over previous
"""TRN2 Bass kernel for nn_BWSGRicker: T=10000-step 4-variable Ricker recurrence.

Strategy: the recurrence s_{k+1} = min(s_k * exp(u(s_k)) + mask_k, 1) is solved
by chunked Picard (waveform relaxation) iteration instead of 10000 serial steps:
  - guess a trajectory for a chunk of steps (zeros / previous iterate),
  - compute all growth factors E_k = exp(u(s_k)) for the chunk vectorized
    (ACT exp for eS, 2 PE matmuls, 2 DVE elementwise, ACT exp),
  - run tensor_tensor_scan (state = min(E*state, 1)) to re-propagate the chunk,
  - repeat until the chunk is a bitwise fixed point, then move on.
The dynamics contract strongly for the target inputs (trajectory collapses to
denormal dust within ~300 steps), so a few sweeps converge the transient
exactly, and the tail is verified in one shot: for the all-zeros tail guess the
growth factors are one constant vector E0 (plus the boundary column), so a
single long scan with a stride-0 broadcast of E0 re-propagates and verifies the
entire tail. Convergence is *proved* on device: each chunk's final sweep writes
to a fresh buffer and max |delta| vs the previous iterate accumulates into a
flag output; flag <= 1e-35 (denormal dust) guarantees the result is the fixed
point of the device step map to within denormal noise. Otherwise the host
re-runs a warm-started refinement program until the flag is clean.

Sharding: a single trajectory is a strict sequential recurrence on 4 scalars --
it cannot be sharded. All 8 cores run the same program (SPMD replication);
core 0's output is returned.
"""
import numpy as np

T = 10000
TAIL_START = 289          # cold-start schedule: full sweeps before, one-shot after
CM = 128                  # max full-sweep chunk width (cold schedule)
CM_WARM = 1024            # max chunk width (warm refinement schedule)
TP = T + 16
NP_OUT = 125              # output staging: 10000 rows = 125 partitions x 80
RPP = 80
CONV_EPS = 1e-35


def _schedule_normal():
    # converges with a sweep of margin in the host model; the device flag
    # check still proves the result regardless.
    return [(1, 129, 7), (129, 289, 3)]


def _schedule_warm():
    sched = [(1, 513, 6)]
    a = 513
    while a < T:
        b = min(a + CM_WARM, T)
        sched.append((a, b, 3))
        a = b
    return sched


def _host_constants(y0, params, logpsi):
    f = np.float32
    y0 = np.asarray(y0, f)
    p = np.asarray(params, f)
    lp = np.asarray(logpsi, f)
    ep0 = np.exp(lp[0], dtype=f)
    ep2 = np.exp(lp[2], dtype=f)
    ep3 = np.exp(lp[3], dtype=f)
    ep4 = np.exp(lp[4], dtype=f)
    ep5 = np.exp(lp[5], dtype=f)
    ep6 = np.exp(lp[6], dtype=f)
    psi1 = lp[1]
    # lane order [S, W, B, G]
    s0 = np.array([y0[2], y0[1], y0[0], y0[3]], f)
    # Wa: [S,W,B,G,1] -> -q   (u = -q*s + (q+r))
    Wa = np.zeros((5, 4), f)
    Wa[3, 0] = -p[0]
    Wa[0, 1] = -p[1]
    Wa[0, 2] = -p[2]
    Wa[1, 2] = -p[2]
    Wa[4, 3] = -ep0
    # Wbc: [S,W,B,G,1, 0*27, eS] -> q + r  (tall K=33; rows 5..31 zeros)
    Wbc = np.zeros((33, 4), f)
    Wbc[3, 0] = p[0]
    Wbc[1, 0] = -ep2
    Wbc[2, 0] = -ep2
    Wbc[4, 0] = -ep3
    Wbc[0, 1] = p[1]
    Wbc[2, 1] = -ep4
    Wbc[4, 1] = -ep5
    Wbc[0, 2] = p[2]
    Wbc[1, 2] = p[2]
    Wbc[4, 2] = -ep6
    Wbc[4, 3] = ep0
    Wbc[32, 3] = -1.0           # -eS into uG
    # mask position: reference masks step j where j == (10.0 + i) - 1.0 in f32
    i_val = f(y0[4])
    jstar = None
    if i_val != 0.0:
        tgt = f(f(f(10.0) + i_val) - f(1.0))
        js = np.arange(1, T, dtype=f)
        hits = np.nonzero(js == tgt)[0]
        if len(hits):
            jstar = int(hits[0]) + 1
    maskv = np.zeros((4, 1), f)
    maskv[2, 0] = f(10.0 / 400.0)  # lane B is index 2
    # growth factors of the all-zeros state, lane order [S,W,B,G]; if all < 1
    # (with margin covering device-matmul ulps) the zero-tail verify chain is
    # monotone decreasing, so only its first column needs to be checked.
    e0 = np.exp(np.array([-ep3, -ep5, -ep6, ep0 - f(1.0)], f), dtype=f)
    return dict(s0=s0, Wa=Wa, Wbc=Wbc, psi1=float(psi1),
                jstar=jstar, maskv=maskv, e0_lt1=bool((e0 < 0.99).all()))


def _build_program(consts, schedule, tail_start):
    import concourse.bacc as bacc
    import concourse.mybir as mybir
    from concourse import tile

    f32 = mybir.dt.float32
    MULT = mybir.AluOpType.mult
    ADD = mybir.AluOpType.add
    SUB = mybir.AluOpType.subtract
    MAXOP = mybir.AluOpType.max
    MIN = mybir.AluOpType.min
    EXP = mybir.ActivationFunctionType.Exp
    X = mybir.AxisListType.X

    jstar = consts["jstar"]
    cm = max(b - a for (a, b, _s) in schedule)
    tail_n = (T - tail_start) if tail_start is not None else 0

    nc = bacc.Bacc("TRN2", target_bir_lowering=False, debug=False, num_devices=8)
    init_in = nc.dram_tensor("binit", [5, TP], f32, kind="ExternalInput").ap()
    wa_in = nc.dram_tensor("wa", [5, 4], f32, kind="ExternalInput").ap()
    wbc_in = nc.dram_tensor("wbc", [33, 4], f32, kind="ExternalInput").ap()
    mask_in = nc.dram_tensor("maskv", [4, 1], f32, kind="ExternalInput").ap()
    y_out = nc.dram_tensor("y", [T, 5], f32, kind="ExternalOutput").ap()
    flag_out = nc.dram_tensor("flag", [4, 1], f32, kind="ExternalOutput").ap()

    with tile.TileContext(nc) as tc:
        with tc.tile_pool(name="p", bufs=1) as pool, \
             tc.tile_pool(name="ps", bufs=1, space="PSUM") as psum_pool:
            # rows 0-3: trajectory [S,W,B,G]; row 4: ones; 5-31: zeros; 32: eS
            Big = pool.tile([33, TP], f32)
            Wa = pool.tile([5, 4], f32)
            Wbc = pool.tile([33, 4], f32)
            MK = pool.tile([4, 1], f32)
            ON1 = pool.tile([4, 1], f32)
            Tq = pool.tile([4, cm], f32)
            Uu = pool.tile([4, cm], f32)
            E = pool.tile([4, cm], f32)
            NewT = pool.tile([4, cm], f32)
            D = pool.tile([4, cm], f32)
            Dm = pool.tile([4, 1], f32)
            CI = pool.tile([4, 1], f32)
            JT = pool.tile([4, 1], f32)
            FlagAcc = pool.tile([4, 1], f32)
            Y4 = pool.tile([NP_OUT, 4 * RPP], f32)
            Yb = pool.tile([NP_OUT, 5 * RPP], f32)
            if tail_n:
                NTT = pool.tile([4, tail_n], f32)
            Vq = psum_pool.tile([4, cm], f32)
            Vqr = psum_pool.tile([4, cm], f32)

            # zero the garbage rows 5..31 over every column the K=33 matmul
            # will read; trajectory zeros beyond come from the binit DMA.
            zspan = max(b for (_a, b, _s) in schedule) + 8
            nc.vector.memset(Big[0:32, 0:zspan], 0.0)
            nc.sync.dma_start(Big[0:5, :], init_in[:])
            nc.sync.dma_start(Wa[:], wa_in[:])
            nc.sync.dma_start(Wbc[:], wbc_in[:])
            nc.sync.dma_start(MK[:], mask_in[:])
            nc.vector.memset(ON1[:], 1.0)
            nc.vector.memset(FlagAcc[:], 0.0)
            nc.vector.memset(Yb[:], 0.0)

            def ones_b(n):
                return ON1[:, 0:1].broadcast_to((4, n))

            def emit_scan(dest, dcol, n, a, Esrc, ecol=0):
                """dest[:, dcol:dcol+n] = scan of steps [a, a+n) with factors
                Esrc[:, ecol:ecol+n]; init from Big col a-1. Mask split."""
                init_ap = Big[0:4, a - 1:a]
                if jstar is not None and a <= jstar < a + n:
                    L1 = jstar - a
                    if L1 > 0:
                        nc.vector.tensor_tensor_scan(
                            dest[0:4, dcol:dcol + L1],
                            Esrc[:, ecol:ecol + L1], ones_b(L1),
                            init_ap, MULT, MIN)
                        prev_ap = dest[0:4, dcol + L1 - 1:dcol + L1]
                    else:
                        prev_ap = init_ap
                    nc.vector.tensor_tensor(CI[:], Esrc[:, ecol + L1:ecol + L1 + 1],
                                            prev_ap, MULT)
                    nc.vector.scalar_tensor_tensor(
                        dest[0:4, dcol + L1:dcol + L1 + 1], CI[:], MK[:],
                        ON1[:, 0:1], ADD, MIN)
                    if L1 + 1 < n:
                        nc.vector.tensor_tensor_scan(
                            dest[0:4, dcol + L1 + 1:dcol + n],
                            Esrc[:, ecol + L1 + 1:ecol + n], ones_b(n - L1 - 1),
                            dest[0:4, dcol + L1:dcol + L1 + 1], MULT, MIN)
                else:
                    nc.vector.tensor_tensor_scan(
                        dest[0:4, dcol:dcol + n], Esrc[:, ecol:ecol + n],
                        ones_b(n), init_ap, MULT, MIN)

            def emit_features(cs, ce):
                """E[:, 0:ce-cs] = growth factors for input-state cols [cs,ce)."""
                n = ce - cs
                nc.scalar.activation(Big[32:33, cs:ce], Big[0:1, cs:ce],
                                     EXP, scale=consts["psi1"])
                for o in range(0, n, 512):
                    e = min(o + 512, n)
                    nc.tensor.matmul(Vq[:, o:e], Wa[:],
                                     Big[0:5, cs + o:cs + e],
                                     start=True, stop=True)
                    nc.tensor.matmul(Vqr[:, o:e], Wbc[:],
                                     Big[0:33, cs + o:cs + e],
                                     start=True, stop=True)
                nc.vector.tensor_tensor(Tq[:, 0:n], Vq[:, 0:n],
                                        Big[0:4, cs:ce], MULT)
                nc.vector.tensor_tensor(Uu[:, 0:n], Tq[:, 0:n],
                                        Vqr[:, 0:n], ADD)
                nc.scalar.activation(E[:, 0:n], Uu[:, 0:n], EXP)
                nc.vector.tensor_copy(JT[:], E[:, 0:1])  # wait-absorbing joiner

            for (a, b, sweeps) in schedule:
                n = b - a
                for s in range(sweeps):
                    last = s == sweeps - 1
                    emit_features(a - 1, b - 1)
                    if not last:
                        nc.vector.tensor_copy(Big[0:4, a:a + 1], E[:, 0:1])
                        emit_scan(Big, a, n, a, E)
                    else:
                        emit_scan(NewT, 0, n, a, E)
                        nc.vector.tensor_tensor(D[:, 0:n], NewT[:, 0:n],
                                                Big[0:4, a:b], SUB)
                        nc.vector.tensor_reduce(Dm[:], D[:, 0:n], X, MAXOP,
                                                apply_absolute_value=True)
                        nc.vector.tensor_tensor(FlagAcc[:], FlagAcc[:], Dm[:],
                                                MAXOP)

            if tail_n:
                # One-shot verification of the all-zeros tail guess: states are
                # zero everywhere except the boundary, so the growth factors
                # are [E_boundary, E0, E0, ...]. Two columns of real feature
                # math give both.
                ts0 = tail_start
                tiny = consts["e0_lt1"] and (jstar is None or jstar < ts0)
                emit_features(ts0 - 1, ts0 + 1)   # E[:,0]=E_ts0, E[:,1]=E0
                # col ts0 result
                nc.vector.tensor_tensor(CI[:], E[:, 0:1],
                                        Big[0:4, ts0 - 1:ts0], MULT)
                if jstar is not None and jstar == ts0:
                    nc.vector.scalar_tensor_tensor(NTT[0:4, 0:1], CI[:], MK[:],
                                                   ON1[:, 0:1], ADD, MIN)
                else:
                    nc.vector.tensor_scalar_min(NTT[0:4, 0:1], CI[:], 1.0)
                if tiny:
                    # E0 < 1 in every lane (host-verified with margin), so the
                    # verify chain min(E0*s, 1) decreases monotonically: its
                    # max |.| is the first column.
                    nc.vector.tensor_reduce(Dm[:], NTT[0:4, 0:1], X, MAXOP,
                                            apply_absolute_value=True)
                else:
                    # full re-propagation with stride-0 E0 factors
                    rem = tail_n - 1
                    a2 = ts0 + 1
                    E0b = E[:, 1:2]
                    init2 = NTT[0:4, 0:1]
                    if jstar is not None and a2 <= jstar < T:
                        L1 = jstar - a2
                        if L1 > 0:
                            nc.vector.tensor_tensor_scan(
                                NTT[0:4, 1:1 + L1], E0b.broadcast_to((4, L1)),
                                ones_b(L1), init2, MULT, MIN)
                            prev_ap = NTT[0:4, L1:L1 + 1]
                        else:
                            prev_ap = init2
                        nc.vector.tensor_tensor(CI[:], E0b, prev_ap, MULT)
                        nc.vector.scalar_tensor_tensor(
                            NTT[0:4, 1 + L1:2 + L1], CI[:], MK[:],
                            ON1[:, 0:1], ADD, MIN)
                        if L1 + 1 < rem:
                            nc.vector.tensor_tensor_scan(
                                NTT[0:4, 2 + L1:tail_n],
                                E0b.broadcast_to((4, rem - L1 - 1)),
                                ones_b(rem - L1 - 1),
                                NTT[0:4, 1 + L1:2 + L1], MULT, MIN)
                    elif rem > 0:
                        nc.vector.tensor_tensor_scan(
                            NTT[0:4, 1:tail_n], E0b.broadcast_to((4, rem)),
                            ones_b(rem), init2, MULT, MIN)
                    nc.vector.tensor_reduce(Dm[:], NTT[0:4, 0:tail_n], X,
                                            MAXOP, apply_absolute_value=True)
                nc.vector.tensor_tensor(FlagAcc[:], FlagAcc[:], Dm[:], MAXOP)

            # output: y[t] = [B, W, S, G, 0] = lanes [2, 1, 0, 3] + zeros.
            # Two-hop rearrangement so every DMA burst is >= 320 bytes.
            for c, lane in ((0, 2), (1, 1), (2, 0), (3, 3)):
                nc.sync.dma_start(Y4[:, RPP * c:RPP * (c + 1)],
                                  Big[lane:lane + 1, 0:T])
            for c in range(4):
                nc.vector.tensor_copy(Yb[:, c:5 * RPP:5],
                                      Y4[:, RPP * c:RPP * (c + 1)])
            nc.sync.dma_start(
                y_out.rearrange("(p r) c -> p (r c)", p=NP_OUT), Yb[:])
            nc.sync.dma_start(flag_out[:], FlagAcc[:])

    nc.finalize()
    return nc


LAST_RESULTS = None  # BassKernelResults of the most recent device run


def _run(consts, schedule, warm_init, tail_start):
    import os
    from concourse.bass_utils import run_bass_kernel_spmd

    nc = _build_program(consts, schedule, tail_start)
    binit = np.zeros((5, TP), np.float32)
    binit[4, :] = 1.0
    if warm_init is not None:
        binit[0:4, 0:T] = warm_init
    binit[0:4, 0] = consts["s0"]
    maskv = consts["maskv"] if consts["jstar"] is not None \
        else np.zeros((4, 1), np.float32)
    im = {"binit": binit, "wa": consts["Wa"], "wbc": consts["Wbc"],
          "maskv": maskv}
    trace = bool(os.environ.get("BWSG_TRACE"))
    r = run_bass_kernel_spmd(nc, [im] * 8, list(range(8)), trace=trace)
    global LAST_RESULTS
    LAST_RESULTS = r
    res = r.results
    y = np.asarray(res[0]["y"])
    flag = float(np.asarray(res[0]["flag"]).max())
    return y, flag


def kernel(y0, params, logpsi):
    consts = _host_constants(y0, params, logpsi)
    y, flag = _run(consts, _schedule_normal(), None, TAIL_START)
    tries = 0
    while flag > CONV_EPS and tries < 8:
        # warm-started refinement until the device proves a fixed point
        warm = y[:, [2, 1, 0, 3]].T.copy()  # back to lane order [S,W,B,G]
        y, flag = _run(consts, _schedule_warm(), warm, None)
        tries += 1
    return y.astype(np.float32)


# revision 19
# speedup vs baseline: 1.1519x; 1.0057x over previous
"""TRN2 Bass kernel for nn_BWSGRicker: T=10000-step 4-variable Ricker recurrence.

Strategy: the recurrence s_{k+1} = min(s_k * exp(u(s_k)) + mask_k, 1) is solved
by chunked Picard (waveform relaxation) iteration instead of 10000 serial steps:
  - guess a trajectory for a chunk of steps (zeros / previous iterate),
  - compute all growth factors E_k = exp(u(s_k)) for the chunk vectorized
    (ACT exp for eS, 2 PE matmuls, 2 DVE elementwise, ACT exp),
  - run tensor_tensor_scan (state = min(E*state, 1)) to re-propagate the chunk,
  - repeat until the chunk is a bitwise fixed point, then move on.
The dynamics contract strongly for the target inputs (trajectory collapses to
denormal dust within ~300 steps), so a few sweeps converge the transient
exactly, and the tail is verified in one shot: for the all-zeros tail guess the
growth factors are one constant vector E0 (plus the boundary column), so a
single long scan with a stride-0 broadcast of E0 re-propagates and verifies the
entire tail. Convergence is *proved* on device: each chunk's final sweep writes
to a fresh buffer and max |delta| vs the previous iterate accumulates into a
flag output; flag <= 1e-35 (denormal dust) guarantees the result is the fixed
point of the device step map to within denormal noise. Otherwise the host
re-runs a warm-started refinement program until the flag is clean.

Sharding: a single trajectory is a strict sequential recurrence on 4 scalars --
it cannot be sharded. All 8 cores run the same program (SPMD replication);
core 0's output is returned.
"""
import numpy as np

T = 10000
TAIL_START = 289          # cold-start schedule: full sweeps before, one-shot after
CM = 128                  # max full-sweep chunk width (cold schedule)
CM_WARM = 1024            # max chunk width (warm refinement schedule)
TP = T + 16
NP_OUT = 125              # output staging: 10000 rows = 125 partitions x 80
RPP = 80
CONV_EPS = 1e-35


def _schedule_normal():
    # converges with a sweep of margin in the host model; the device flag
    # check still proves the result regardless.
    return [(1, 129, 7), (129, 289, 3)]


def _schedule_warm():
    sched = [(1, 513, 6)]
    a = 513
    while a < T:
        b = min(a + CM_WARM, T)
        sched.append((a, b, 3))
        a = b
    return sched


def _host_constants(y0, params, logpsi):
    f = np.float32
    y0 = np.asarray(y0, f)
    p = np.asarray(params, f)
    lp = np.asarray(logpsi, f)
    ep0 = np.exp(lp[0], dtype=f)
    ep2 = np.exp(lp[2], dtype=f)
    ep3 = np.exp(lp[3], dtype=f)
    ep4 = np.exp(lp[4], dtype=f)
    ep5 = np.exp(lp[5], dtype=f)
    ep6 = np.exp(lp[6], dtype=f)
    psi1 = lp[1]
    # lane order [S, W, B, G]
    s0 = np.array([y0[2], y0[1], y0[0], y0[3]], f)
    # Wa: [S,W,B,G,1] -> -q   (u = -q*s + (q+r))
    Wa = np.zeros((5, 4), f)
    Wa[3, 0] = -p[0]
    Wa[0, 1] = -p[1]
    Wa[0, 2] = -p[2]
    Wa[1, 2] = -p[2]
    Wa[4, 3] = -ep0
    # Wbc: [S,W,B,G,1, 0*27, eS] -> q + r  (tall K=33; rows 5..31 zeros)
    Wbc = np.zeros((33, 4), f)
    Wbc[3, 0] = p[0]
    Wbc[1, 0] = -ep2
    Wbc[2, 0] = -ep2
    Wbc[4, 0] = -ep3
    Wbc[0, 1] = p[1]
    Wbc[2, 1] = -ep4
    Wbc[4, 1] = -ep5
    Wbc[0, 2] = p[2]
    Wbc[1, 2] = p[2]
    Wbc[4, 2] = -ep6
    Wbc[4, 3] = ep0
    Wbc[32, 3] = -1.0           # -eS into uG
    # mask position: reference masks step j where j == (10.0 + i) - 1.0 in f32
    i_val = f(y0[4])
    jstar = None
    if i_val != 0.0:
        tgt = f(f(f(10.0) + i_val) - f(1.0))
        js = np.arange(1, T, dtype=f)
        hits = np.nonzero(js == tgt)[0]
        if len(hits):
            jstar = int(hits[0]) + 1
    maskv = np.zeros((4, 1), f)
    maskv[2, 0] = f(10.0 / 400.0)  # lane B is index 2
    # growth factors of the all-zeros state, lane order [S,W,B,G]; if all < 1
    # (with margin covering device-matmul ulps) the zero-tail verify chain is
    # monotone decreasing, so only its first column needs to be checked.
    e0 = np.exp(np.array([-ep3, -ep5, -ep6, ep0 - f(1.0)], f), dtype=f)
    return dict(s0=s0, Wa=Wa, Wbc=Wbc, psi1=float(psi1),
                jstar=jstar, maskv=maskv, e0_lt1=bool((e0 < 0.99).all()))


def _build_program(consts, schedule, tail_start):
    import concourse.bacc as bacc
    import concourse.mybir as mybir
    from concourse import tile

    f32 = mybir.dt.float32
    MULT = mybir.AluOpType.mult
    ADD = mybir.AluOpType.add
    SUB = mybir.AluOpType.subtract
    MAXOP = mybir.AluOpType.max
    MIN = mybir.AluOpType.min
    EXP = mybir.ActivationFunctionType.Exp
    X = mybir.AxisListType.X

    jstar = consts["jstar"]
    cm = max(b - a for (a, b, _s) in schedule)
    tail_n = (T - tail_start) if tail_start is not None else 0

    nc = bacc.Bacc("TRN2", target_bir_lowering=False, debug=False, num_devices=8)
    init_in = nc.dram_tensor("binit", [5, TP], f32, kind="ExternalInput").ap()
    wa_in = nc.dram_tensor("wa", [5, 4], f32, kind="ExternalInput").ap()
    wbc_in = nc.dram_tensor("wbc", [33, 4], f32, kind="ExternalInput").ap()
    mask_in = nc.dram_tensor("maskv", [4, 1], f32, kind="ExternalInput").ap()
    y_out = nc.dram_tensor("y", [T, 5], f32, kind="ExternalOutput").ap()
    flag_out = nc.dram_tensor("flag", [4, 1], f32, kind="ExternalOutput").ap()

    with tile.TileContext(nc) as tc:
        with tc.tile_pool(name="p", bufs=1) as pool, \
             tc.tile_pool(name="ps", bufs=1, space="PSUM") as psum_pool:
            # rows 0-3: trajectory [S,W,B,G]; row 4: ones; 5-31: zeros; 32: eS
            Big = pool.tile([33, TP], f32)
            Wa = pool.tile([5, 4], f32)
            Wbc = pool.tile([33, 4], f32)
            MK = pool.tile([4, 1], f32)
            ON1 = pool.tile([4, 1], f32)
            Tq = pool.tile([4, cm], f32)
            Uu = pool.tile([4, cm], f32)
            E = pool.tile([4, cm], f32)
            NewT = pool.tile([4, cm], f32)
            D = pool.tile([4, cm], f32)
            Dm = pool.tile([4, 1], f32)
            CI = pool.tile([4, 1], f32)
            JT = pool.tile([4, 1], f32)
            FlagAcc = pool.tile([4, 1], f32)
            Y4 = pool.tile([NP_OUT, 4 * RPP], f32)
            Yb = pool.tile([NP_OUT, 5 * RPP], f32)
            if tail_n:
                NTT = pool.tile([4, tail_n], f32)
            Vq = psum_pool.tile([4, cm], f32)
            Vqr = psum_pool.tile([4, cm], f32)

            # zero the garbage rows 5..31 over every column the K=33 matmul
            # will read; trajectory zeros beyond come from the binit DMA.
            zspan = max(b for (_a, b, _s) in schedule) + 8
            nc.vector.memset(Big[0:32, 0:zspan], 0.0)
            nc.sync.dma_start(Big[0:5, :], init_in[:])
            nc.sync.dma_start(Wa[:], wa_in[:])
            nc.sync.dma_start(Wbc[:], wbc_in[:])
            nc.sync.dma_start(MK[:], mask_in[:])
            nc.vector.memset(ON1[:], 1.0)
            nc.vector.memset(FlagAcc[:], 0.0)
            nc.vector.memset(Yb[:], 0.0)
            # dummy exp: pulls the ~1.3us ACT table load into the DMA window
            nc.scalar.activation(JT[:], ON1[:], EXP)

            def ones_b(n):
                return ON1[:, 0:1].broadcast_to((4, n))

            def emit_scan(dest, dcol, n, a, Esrc, ecol=0):
                """dest[:, dcol:dcol+n] = scan of steps [a, a+n) with factors
                Esrc[:, ecol:ecol+n]; init from Big col a-1. Mask split."""
                init_ap = Big[0:4, a - 1:a]
                if jstar is not None and a <= jstar < a + n:
                    L1 = jstar - a
                    if L1 > 0:
                        nc.vector.tensor_tensor_scan(
                            dest[0:4, dcol:dcol + L1],
                            Esrc[:, ecol:ecol + L1], ones_b(L1),
                            init_ap, MULT, MIN)
                        prev_ap = dest[0:4, dcol + L1 - 1:dcol + L1]
                    else:
                        prev_ap = init_ap
                    nc.vector.tensor_tensor(CI[:], Esrc[:, ecol + L1:ecol + L1 + 1],
                                            prev_ap, MULT)
                    nc.vector.scalar_tensor_tensor(
                        dest[0:4, dcol + L1:dcol + L1 + 1], CI[:], MK[:],
                        ON1[:, 0:1], ADD, MIN)
                    if L1 + 1 < n:
                        nc.vector.tensor_tensor_scan(
                            dest[0:4, dcol + L1 + 1:dcol + n],
                            Esrc[:, ecol + L1 + 1:ecol + n], ones_b(n - L1 - 1),
                            dest[0:4, dcol + L1:dcol + L1 + 1], MULT, MIN)
                else:
                    nc.vector.tensor_tensor_scan(
                        dest[0:4, dcol:dcol + n], Esrc[:, ecol:ecol + n],
                        ones_b(n), init_ap, MULT, MIN)

            def emit_features(cs, ce):
                """E[:, 0:ce-cs] = growth factors for input-state cols [cs,ce)."""
                n = ce - cs
                nc.scalar.activation(Big[32:33, cs:ce], Big[0:1, cs:ce],
                                     EXP, scale=consts["psi1"])
                for o in range(0, n, 512):
                    e = min(o + 512, n)
                    nc.tensor.matmul(Vq[:, o:e], Wa[:],
                                     Big[0:5, cs + o:cs + e],
                                     start=True, stop=True)
                    nc.tensor.matmul(Vqr[:, o:e], Wbc[:],
                                     Big[0:33, cs + o:cs + e],
                                     start=True, stop=True)
                nc.vector.tensor_tensor(Tq[:, 0:n], Vq[:, 0:n],
                                        Big[0:4, cs:ce], MULT)
                nc.vector.tensor_tensor(Uu[:, 0:n], Tq[:, 0:n],
                                        Vqr[:, 0:n], ADD)
                nc.scalar.activation(E[:, 0:n], Uu[:, 0:n], EXP)
                nc.vector.tensor_copy(JT[:], E[:, 0:1])  # wait-absorbing joiner

            for (a, b, sweeps) in schedule:
                n = b - a
                for s in range(sweeps):
                    last = s == sweeps - 1
                    emit_features(a - 1, b - 1)
                    if not last:
                        nc.vector.tensor_copy(Big[0:4, a:a + 1], E[:, 0:1])
                        emit_scan(Big, a, n, a, E)
                    else:
                        emit_scan(NewT, 0, n, a, E)
                        nc.vector.tensor_tensor(D[:, 0:n], NewT[:, 0:n],
                                                Big[0:4, a:b], SUB)
                        nc.vector.tensor_reduce(Dm[:], D[:, 0:n], X, MAXOP,
                                                apply_absolute_value=True)
                        nc.vector.tensor_tensor(FlagAcc[:], FlagAcc[:], Dm[:],
                                                MAXOP)

            if tail_n:
                # One-shot verification of the all-zeros tail guess: states are
                # zero everywhere except the boundary, so the growth factors
                # are [E_boundary, E0, E0, ...]. Two columns of real feature
                # math give both.
                ts0 = tail_start
                tiny = consts["e0_lt1"] and (jstar is None or jstar < ts0)
                emit_features(ts0 - 1, ts0 + 1)   # E[:,0]=E_ts0, E[:,1]=E0
                # col ts0 result
                nc.vector.tensor_tensor(CI[:], E[:, 0:1],
                                        Big[0:4, ts0 - 1:ts0], MULT)
                if jstar is not None and jstar == ts0:
                    nc.vector.scalar_tensor_tensor(NTT[0:4, 0:1], CI[:], MK[:],
                                                   ON1[:, 0:1], ADD, MIN)
                else:
                    nc.vector.tensor_scalar_min(NTT[0:4, 0:1], CI[:], 1.0)
                if tiny:
                    # E0 < 1 in every lane (host-verified with margin), so the
                    # verify chain min(E0*s, 1) decreases monotonically: its
                    # max |.| is the first column.
                    nc.vector.tensor_reduce(Dm[:], NTT[0:4, 0:1], X, MAXOP,
                                            apply_absolute_value=True)
                else:
                    # full re-propagation with stride-0 E0 factors
                    rem = tail_n - 1
                    a2 = ts0 + 1
                    E0b = E[:, 1:2]
                    init2 = NTT[0:4, 0:1]
                    if jstar is not None and a2 <= jstar < T:
                        L1 = jstar - a2
                        if L1 > 0:
                            nc.vector.tensor_tensor_scan(
                                NTT[0:4, 1:1 + L1], E0b.broadcast_to((4, L1)),
                                ones_b(L1), init2, MULT, MIN)
                            prev_ap = NTT[0:4, L1:L1 + 1]
                        else:
                            prev_ap = init2
                        nc.vector.tensor_tensor(CI[:], E0b, prev_ap, MULT)
                        nc.vector.scalar_tensor_tensor(
                            NTT[0:4, 1 + L1:2 + L1], CI[:], MK[:],
                            ON1[:, 0:1], ADD, MIN)
                        if L1 + 1 < rem:
                            nc.vector.tensor_tensor_scan(
                                NTT[0:4, 2 + L1:tail_n],
                                E0b.broadcast_to((4, rem - L1 - 1)),
                                ones_b(rem - L1 - 1),
                                NTT[0:4, 1 + L1:2 + L1], MULT, MIN)
                    elif rem > 0:
                        nc.vector.tensor_tensor_scan(
                            NTT[0:4, 1:tail_n], E0b.broadcast_to((4, rem)),
                            ones_b(rem), init2, MULT, MIN)
                    nc.vector.tensor_reduce(Dm[:], NTT[0:4, 0:tail_n], X,
                                            MAXOP, apply_absolute_value=True)
                nc.vector.tensor_tensor(FlagAcc[:], FlagAcc[:], Dm[:], MAXOP)

            # output: y[t] = [B, W, S, G, 0] = lanes [2, 1, 0, 3] + zeros.
            # Two-hop rearrangement so every DMA burst is >= 320 bytes; the
            # four lane DMAs go out on different engine queues in parallel.
            dma_engs = (nc.sync, nc.gpsimd, nc.scalar, nc.sync)
            for (c, lane), eng in zip(((0, 2), (1, 1), (2, 0), (3, 3)),
                                      dma_engs):
                eng.dma_start(Y4[:, RPP * c:RPP * (c + 1)],
                              Big[lane:lane + 1, 0:T])
            for c in range(4):
                nc.vector.tensor_copy(Yb[:, c:5 * RPP:5],
                                      Y4[:, RPP * c:RPP * (c + 1)])
            nc.sync.dma_start(
                y_out.rearrange("(p r) c -> p (r c)", p=NP_OUT), Yb[:])
            nc.sync.dma_start(flag_out[:], FlagAcc[:])

    nc.finalize()
    return nc


LAST_RESULTS = None  # BassKernelResults of the most recent device run


def _run(consts, schedule, warm_init, tail_start):
    import os
    from concourse.bass_utils import run_bass_kernel_spmd

    nc = _build_program(consts, schedule, tail_start)
    binit = np.zeros((5, TP), np.float32)
    binit[4, :] = 1.0
    if warm_init is not None:
        binit[0:4, 0:T] = warm_init
    binit[0:4, 0] = consts["s0"]
    maskv = consts["maskv"] if consts["jstar"] is not None \
        else np.zeros((4, 1), np.float32)
    im = {"binit": binit, "wa": consts["Wa"], "wbc": consts["Wbc"],
          "maskv": maskv}
    trace = bool(os.environ.get("BWSG_TRACE"))
    r = run_bass_kernel_spmd(nc, [im] * 8, list(range(8)), trace=trace)
    global LAST_RESULTS
    LAST_RESULTS = r
    res = r.results
    y = np.asarray(res[0]["y"])
    flag = float(np.asarray(res[0]["flag"]).max())
    return y, flag


def kernel(y0, params, logpsi):
    consts = _host_constants(y0, params, logpsi)
    y, flag = _run(consts, _schedule_normal(), None, TAIL_START)
    tries = 0
    while flag > CONV_EPS and tries < 8:
        # warm-started refinement until the device proves a fixed point
        warm = y[:, [2, 1, 0, 3]].T.copy()  # back to lane order [S,W,B,G]
        y, flag = _run(consts, _schedule_warm(), warm, None)
        tries += 1
    return y.astype(np.float32)
